# revision 39
# baseline (speedup 1.0000x reference)
"""Trainium2 Bass kernel for causal multi-head attention + output projection.

Problem (hardcoded): x[4, 2048, 1024] fp32, 16 heads, head_dim 64, causal,
torch-Linear convention (y = x @ W.T), output projection with bias.

Sharding over 8 NeuronCores: batch (4) x head-group (2 groups of 8 heads).
Core c = (b, g): computes q/k/v for heads [8g, 8g+8) of batch b, causal
attention in the S^T layout (keys on partitions, queries on free dim), a
partial output projection over its own 512 O-dims for all 2048 queries, and
a pairwise ReduceScatter(add) scattered along the OUTPUT-D dimension: core
even ends with the final y[:, 0:512], core odd with y[:, 512:1024], for all
2048 rows.  The host concatenates along D.

Structure (vs the pair-major baseline at 376us; this version ~368us):
  - attention runs QUERY-BLOCK-major (qb outer, pair inner), so after each
    qb all 4 pairs' o_keep rows for that qb exist and the projection + two
    256-row ReduceScatter chunks launch immediately -> 6 of the 8
    collectives hide under later attention (the old version serialized
    ~100us of collectives at the end).
  - input DMAs are batched into 0.5-1MB transfers split across the sync
    (x, token-block chunks) and gpsimd (weights) queues; the old per-128KB
    chunks ran at ~180GB/s on one queue and the ones-column scatter DMA
    (4096 2-byte descriptors, ~34us!) is now a gpsimd memset.
  - PSUM->SBUF evictions alternate Scalar/Vector by parity; the o_keep
    eviction + denominator-row copy ride Scalar (PSUM-close, short queue)
    so the o psum frees fast; projection bias is folded into the matmul
    accumulation group as a K=1 ones-x-bias-row rank-1 update.
  - QKV emission chain is token-block-major (th outer, pair inner) to feed
    the qb-major attention order; projection entries are spliced into the
    chain at the current drain position so they emit during later work.

Measured (HW traces): PE busy ~275us (the critical engine; ~14 GFLOP bf16
vs 78.6 TF/s peak => ~178us ideal + AV's 65/128-partition denominator tax
+ LDWEIGHTS + ~40us HAM cold-clock), ACT/exp ~155-210us, DVE ~80-145us,
8x ReduceScatter ~15us each on a 26GB/s bus.  Rejected experiments: fp8
anywhere (sim rel-err 2.4-5e-2 > 2e-2 gate), 256-wide attention blocks
(HAM oscillation + peer-skewed collectives, +90us), PE-side causal mask
via maskneg@I accumulation (+25us, breaks S-pair co-execution), feeder
front-loading (starves the PE queue head on WAR deps).

Attention per (pair, qb): the two heads share one 2-bank PSUM tile for S^T
(head at col 0 / 512 -> different banks), the two row-tiled (64x128) S
matmuls co-execute on PE tiles (0,0)/(64,0), and the softmax exp for both
heads is ONE ACT instruction on a strided [128, 2, n] view.  Softmax
denominators ride as a ones-column in V (row 64 of the O psum);
normalization = reciprocal + partition-broadcast via a small DRAM bounce on
the gpsimd queue, off the critical path.

All matmul operands are bf16 (~0.5% rel err, same PE throughput as fp32r,
half the DMA/SBUF/collective traffic).  PSUM accumulation is fp32.
"""
import sys
import types
from contextlib import ExitStack

import numpy as np

import concourse.bass as bass
import concourse.mybir as mybir
import concourse.tile as tile
from concourse import bacc, bass_utils

F32 = mybir.dt.float32
BF16 = mybir.dt.bfloat16
AF = mybir.ActivationFunctionType
OP = mybir.AluOpType

import os as _os
_AV_SPLIT = bool(int(_os.environ.get("ATTN_AV_SPLIT", "1")))
_JBATCH = int(_os.environ.get("ATTN_JBATCH", "2"))

B, T, D = 4, 2048, 1024
HG = 8           # heads per core
NP = 4           # head pairs per core
QB = 512         # query block
NQB = T // QB    # 4 query blocks
N_CORES = 8
SCALE = 1.0 / 8.0
MODE = "rs"  # harness compat


# ---------------------------------------------------------------------------
# environment glue
# ---------------------------------------------------------------------------

def _install_ntff_hook():
    if 'antenv.axon_hooks' in sys.modules:
        return
    try:
        from trn_agent_boot.trn_boot import _ntff_profile_via_ctypes
        hook = _ntff_profile_via_ctypes('/opt/axon/libaxon_pjrt.so')
    except Exception:
        hook = None
    mod = types.ModuleType('antenv.axon_hooks')
    mod.get_axon_ntff_profile_hook = lambda: hook
    mod.set_axon_ntff_profile_hook = lambda h: None
    sys.modules['antenv.axon_hooks'] = mod


def _run_spmd(nc, in_maps, trace=False):
    from concourse.bass_interp import get_hw_module
    bass_utils.upload_artifacts = lambda tmpdir: tmpdir
    if trace:
        _install_ntff_hook()
    old_m = nc.m
    nc.m = get_hw_module(nc.m)
    try:
        return bass_utils.run_bass_kernel_spmd(
            nc, in_maps, core_ids=list(range(N_CORES)),
            trace=trace, trace_cores=[0] if trace else None,
        )
    finally:
        nc.m = old_m


# ---------------------------------------------------------------------------
# kernel program
# ---------------------------------------------------------------------------

def build_nc():
    nc = bacc.Bacc("TRN2", target_bir_lowering=False, debug=False,
                   enable_asserts=False, num_devices=N_CORES)
    xT = nc.dram_tensor("xT", [D, T], BF16, kind="ExternalInput").ap()
    wqT = nc.dram_tensor("wqT", [D, 512], BF16, kind="ExternalInput").ap()
    wkT = nc.dram_tensor("wkT", [D, 512], BF16, kind="ExternalInput").ap()
    wvT = nc.dram_tensor("wvT", [D, 512], BF16, kind="ExternalInput").ap()
    wpT = nc.dram_tensor("wpT", [512, D], BF16, kind="ExternalInput").ap()
    bias = nc.dram_tensor("bias", [1, D], BF16, kind="ExternalInput").ap()
    mask = nc.dram_tensor("mask", [128, 128], BF16, kind="ExternalInput").ap()
    snum = nc.dram_tensor("snum", [20, 2 * QB], F32).ap()
    srecd = nc.dram_tensor("srecd", [20, 2 * QB], F32).ap()
    # y_part[c] = partial y rows [256c, +256) split into the two D-halves
    # (scatter dim first) so each ReduceScatter chunk is contiguous
    y_part = nc.dram_tensor("y_part", [T // 256, 2, 256, 512], BF16).ap()
    yred = nc.dram_tensor("yred", [T, 512], BF16).ap()
    yout = nc.dram_tensor("yout", [T, 512], BF16, kind="ExternalOutput").ap()

    with tile.TileContext(nc) as tc, ExitStack() as ctx:
        per = ctx.enter_context(tc.tile_pool(name="per", bufs=1))

        mask_sb = per.tile([128, 128], BF16, tag="mask")
        qT_sb = per.tile([128, NP, T], BF16, tag="qT")
        kT_sb = per.tile([128, NP, T], BF16, tag="kT")
        o_keep = per.tile([128, NP, T], BF16, tag="okeep")
        bias_sb = per.tile([1, D], BF16, tag="brow")
        ones_sb = per.tile([1, 128], BF16, tag="ones")
        wp_sb = per.tile([128, NP, D], BF16, tag="wp")
        wk_sb = per.tile([128, 8, 512], BF16, tag="wk")
        wq_sb = per.tile([128, 8, 512], BF16, tag="wq")
        wv_sb = per.tile([128, 8, 512], BF16, tag="wv")
        x_th = [per.tile([128, 8, QB], BF16, tag=f"x{th}", name=f"x_th{th}")
                for th in range(NQB)]

        xT_r = xT.rearrange("(ko ki) t -> ki ko t", ki=128)

        # batched input loads: first-consumed first; x in token-block
        # chunks on sync (the first attention block only needs tokens
        # 0:512), weights on gpsimd, so the queues stream in parallel
        wkT_r = wkT.rearrange("(ko ki) n -> ki ko n", ki=128)
        nc.gpsimd.dma_start(wk_sb[:, 0:4], wkT_r[:, 0:4])
        nc.sync.dma_start(x_th[0][:, 0:4], xT_r[:, 0:4, 0:QB])
        nc.gpsimd.dma_start(wk_sb[:, 4:8], wkT_r[:, 4:8])
        nc.sync.dma_start(x_th[0][:, 4:8], xT_r[:, 4:8, 0:QB])
        nc.gpsimd.dma_start(wq_sb[:],
                            wqT.rearrange("(ko ki) n -> ki ko n", ki=128))
        nc.sync.dma_start(mask_sb[:], mask[:])
        nc.sync.dma_start(bias_sb[:], bias[:])
        nc.gpsimd.dma_start(wv_sb[:],
                            wvT.rearrange("(ko ki) n -> ki ko n", ki=128))
        for th in range(1, NQB):
            nc.sync.dma_start(x_th[th][:], xT_r[:, :, th * QB:(th + 1) * QB])
        nc.gpsimd.dma_start(wp_sb[:],
                            wpT.rearrange("(ko ki) n -> ki ko n", ki=128))
        nc.gpsimd.memset(ones_sb[:], 1.0)

        def xh(kk, th):
            return x_th[th][:, kk, :]

        with ExitStack() as attn_ctx:
            vpool = attn_ctx.enter_context(tc.tile_pool(name="vpool", bufs=4))
            qkps = attn_ctx.enter_context(
                tc.tile_pool(name="qkps", bufs=2, space="PSUM"))
            sps = attn_ctx.enter_context(
                tc.tile_pool(name="sps", bufs=2, space="PSUM"))
            ops = attn_ctx.enter_context(
                tc.tile_pool(name="ops", bufs=2, space="PSUM"))
            epool = attn_ctx.enter_context(tc.tile_pool(name="epool", bufs=3))
            npool = attn_ctx.enter_context(tc.tile_pool(name="npool", bufs=4))
            ypool = attn_ctx.enter_context(tc.tile_pool(name="ypool", bufs=3))

            v_tiles = {}

            # ----------------------------------------------------------
            # background QKV emission chain, token-block-major: after
            # marker (p, th), pair p's q/k/v for tokens up to 512*(th+1)
            # are fully emitted.
            # ----------------------------------------------------------
            def qkv_chain():
                chain = []
                for th in range(NQB):
                    for p in range(NP):
                        if th == 0:
                            # ones column (softmax denominator rows of the
                            # AV psum) via engine memset -- a scatter DMA
                            # here costs 4096 2-byte descriptors (~34us!)
                            def ones_set(p=p):
                                v_sb = vpool.tile([128, 16, 2, 65], BF16,
                                                  tag="v", name=f"v{p}")
                                v_tiles[p] = v_sb
                                nc.gpsimd.memset(v_sb[:, :, :, 64], 1.0)
                            chain.append((ones_set, None))
                        for wsb, dst in ((wk_sb, kT_sb), (wq_sb, qT_sb)):
                            box = {}

                            def fill(half, box=box, wsb=wsb, th=th, p=p):
                                if half == 0:
                                    box["pt"] = qkps.tile([128, QB], F32,
                                                          tag="pt", name="pt")
                                pt = box["pt"]
                                for kk in range(4 * half, 4 * half + 4):
                                    nc.tensor.matmul(
                                        pt[:],
                                        lhsT=wsb[:, kk, p * 128:(p + 1) * 128],
                                        rhs=xh(kk, th),
                                        start=(kk == 0), stop=(kk == 7))

                            def evict(box=box, dst=dst, th=th, p=p):
                                # alternate the eviction engine so neither
                                # queue's backlog gates the S-matmul chain
                                eng = nc.scalar if (p + th) % 2 else nc.vector
                                if eng is nc.scalar:
                                    nc.scalar.copy(
                                        dst[:, p, th * QB:(th + 1) * QB],
                                        box["pt"][:])
                                else:
                                    nc.vector.tensor_copy(
                                        dst[:, p, th * QB:(th + 1) * QB],
                                        box["pt"][:])
                            chain.append((lambda f=fill: f(0), None))
                            chain.append((lambda f=fill: f(1), None))
                            chain.append((evict, None))
                        # V for key blocks 4*th .. 4*th+3 (token-major)
                        box = {}

                        def vfill(sub, box=box, th=th, p=p):
                            if sub == 0:
                                box["pt"] = qkps.tile([128, QB], F32,
                                                      tag="pt", name="pt")
                            pt = box["pt"]
                            for kk in range(8):
                                nc.tensor.matmul(
                                    pt[:, sub * 128:(sub + 1) * 128],
                                    lhsT=xh(kk, th)[:,
                                                    sub * 128:(sub + 1) * 128],
                                    rhs=wv_sb[:, kk, p * 128:(p + 1) * 128],
                                    start=(kk == 0), stop=(kk == 7))

                        def vevict(box=box, th=th, p=p):
                            if (p + th) % 2:
                                nc.vector.tensor_copy(
                                    v_tiles[p][:, 4 * th:4 * th + 4, :, 0:64],
                                    box["pt"][:].rearrange(
                                        "q (m h d) -> q m h d", m=4, h=2))
                            else:
                                nc.scalar.copy(
                                    v_tiles[p][:, 4 * th:4 * th + 4, :, 0:64],
                                    box["pt"][:].rearrange(
                                        "q (m h d) -> q m h d", m=4, h=2))
                        for sub in range(4):
                            chain.append((lambda f=vfill, s=sub: f(s), None))
                        chain.append((vevict, (p, th)))
                return chain

            chain = qkv_chain()
            pos = [0]
            emitted = {}

            def emit_next():
                if pos[0] >= len(chain):
                    return False
                fn, marker = chain[pos[0]]
                pos[0] += 1
                fn()
                if marker is not None:
                    emitted[marker[0]] = marker[1]
                return True

            def drain_until(p, th):
                while emitted.get(p, -1) < th:
                    if not emit_next():
                        raise RuntimeError("qkv chain exhausted early")

            def feeder(k):
                for _ in range(k):
                    if not emit_next():
                        return

            # ----------------------------------------------------------
            # attention + normalization, over generalized query blocks
            # (q0, qw): qb0-2 run 512-wide; the LAST 512 queries run as
            # two 256-wide blocks so the final projection + ReduceScatter
            # chunk shrinks (the exposed tail halves).
            # ----------------------------------------------------------
            BLOCKS = [(0, QB), (QB, QB), (2 * QB, QB), (3 * QB, QB)]

            def attend_block(p, bi):
                q0, qw = BLOCKS[bi]
                jmax = (q0 + qw) // 128
                o_ps = [ops.tile([65, QB], F32, tag="o", name=f"o{hl}")
                        for hl in range(2)]
                for j0 in range(0, jmax, _JBATCH):
                    batch = range(j0, min(j0 + _JBATCH, jmax))
                    s_tiles = {}
                    e_tiles = {}
                    for j in batch:
                        qs = max(0, 128 * j - q0)
                        s_t = sps.tile([128, 2 * QB], F32, tag="s",
                                       name=f"s{j}")
                        s_tiles[j] = s_t
                        for hl in range(2):
                            pb = 64 * hl
                            # head hl at column hl*QB: each head's S stays
                            # in its OWN PSUM bank (start=True clears the
                            # whole bank's has_written bits)
                            nc.tensor.matmul(
                                s_t[:, hl * QB + qs:hl * QB + qw],
                                lhsT=kT_sb[pb:pb + 64, p,
                                           j * 128:(j + 1) * 128],
                                rhs=qT_sb[pb:pb + 64, p,
                                          q0 + qs:q0 + qw],
                                start=True, stop=True)
                    for j in batch:
                        qs = max(0, 128 * j - q0)
                        e_t = epool.tile([128, 2, QB], BF16, tag="e",
                                         name=f"e{j}")
                        e_tiles[j] = e_t
                        s_v = s_tiles[j].rearrange("q (h n) -> q h n", h=2)
                        nc.scalar.activation(e_t[:, :, qs:qw],
                                             s_v[:, :, qs:qw],
                                             AF.Exp, scale=SCALE)
                        if 128 * j >= q0:
                            nc.vector.tensor_tensor(
                                e_t[:, :, qs:qs + 128],
                                e_t[:, :, qs:qs + 128],
                                mask_sb[:, None, :]
                                .broadcast_to([128, 2, 128]),
                                OP.mult)
                    feeder(1)
                    for j in batch:
                        qs = max(0, 128 * j - q0)
                        e_t = e_tiles[j]
                        last = (j == jmax - 1)
                        for hl in range(2):
                            if 128 * j >= q0 and _AV_SPLIT and j > 0:
                                if qs + 128 < qw:
                                    nc.tensor.matmul(
                                        o_ps[hl][:, qs + 128:qw],
                                        lhsT=v_tiles[p][:, j, hl, :],
                                        rhs=e_t[:, hl, qs + 128:qw],
                                        start=(j == 0), stop=False,
                                        skip_group_check=True)
                                nc.tensor.matmul(
                                    o_ps[hl][:, qs:qs + 128],
                                    lhsT=v_tiles[p][:, j, hl, :],
                                    rhs=e_t[:, hl, qs:qs + 128],
                                    start=(j == 0), stop=last,
                                    skip_group_check=True)
                            else:
                                nc.tensor.matmul(
                                    o_ps[hl][:, qs:qw],
                                    lhsT=v_tiles[p][:, j, hl, :],
                                    rhs=e_t[:, hl, qs:qw],
                                    start=(j == 0), stop=last,
                                    skip_group_check=True)
                    feeder(1)
                return o_ps

            def finish_pair_blk(p, bi, o_ps):
                """Evict + normalize both heads of the pair for this query
                block.  Denominators of both heads ride ONE DRAM bounce:
                write [2, qw], reload spread as [64, 2qw/64], reciprocal
                (few cols -> fast), write back, one broadcast load for both
                partition halves.  DMAs go on the gpsimd queue to keep the
                other queues clear."""
                q0, qw = BLOCKS[bi]
                row = p * len(BLOCKS) + bi
                stmp = npool.tile([1, 2 * QB], F32, tag="st", name="stmp")
                for hl in range(2):
                    # denominator row + o eviction on SCALAR: releases the
                    # o_ps psum fast (short queue; ScalarE is PSUM-close)
                    nc.scalar.copy(stmp[0:1, hl * qw:(hl + 1) * qw],
                                   o_ps[hl][64:65, 0:qw])
                nc.gpsimd.dma_start(snum[row:row + 1, 0:2 * qw],
                                    stmp[0:1, 0:2 * qw])
                st64 = npool.tile([64, 2 * QB // 64], F32, tag="sp",
                                  name="st64")
                nb = 2 * qw // 64
                nc.gpsimd.dma_start(
                    st64[:, 0:nb],
                    snum[row, 0:2 * qw].rearrange("(a b) -> a b", a=64))
                nc.vector.reciprocal(st64[:, 0:nb], st64[:, 0:nb])
                nc.gpsimd.dma_start(
                    srecd[row, 0:2 * qw].rearrange("(a b) -> a b", a=64),
                    st64[:, 0:nb])
                bcr = npool.tile([128, QB], F32, tag="bcr", name="bcr")
                for hl in range(2):
                    pb = 64 * hl
                    nc.gpsimd.dma_start(
                        bcr[pb:pb + 64, 0:qw],
                        srecd[row][None, hl * qw:(hl + 1) * qw]
                        .broadcast_to([64, qw]))
                    dst = o_keep[pb:pb + 64, p, q0:q0 + qw]
                    nc.scalar.copy(dst, o_ps[hl][0:64, 0:qw])
                    nc.vector.tensor_tensor(dst, dst, bcr[pb:pb + 64, 0:qw],
                                            OP.mult)

            # ----------------------------------------------------------
            # projection for a query block: y rows [q0, q0+qw), all 1024
            # output cols, then pairwise ReduceScatter along D in 256-row
            # chunks.  Needs o_keep of ALL pairs for the block -> entries
            # spliced into the chain right after the block's last pair.
            # ----------------------------------------------------------
            def rs_out(c):
                nc.gpsimd.collective_compute(
                    "ReduceScatter", OP.add,
                    replica_groups=[[0, 1], [2, 3], [4, 5], [6, 7]],
                    ins=[y_part[c]],
                    outs=[yred[c * 256:(c + 1) * 256, :]],
                )
                nc.sync.dma_start(yout[c * 256:(c + 1) * 256, :],
                                  yred[c * 256:(c + 1) * 256, :])

            def proj_blk_entries(bi):
                q0, qw = BLOCKS[bi]
                entries = []
                for mi in range(qw // 128):
                    m = q0 // 128 + mi

                    def tile_work(m=m):
                        y_sb = ypool.tile([128, D], BF16, tag="y",
                                          name="y_sb")
                        for nch in range(2):
                            sl = slice(nch * 512, (nch + 1) * 512)
                            yp = qkps.tile([128, QB], F32, tag="pt",
                                           name="yp")
                            for kk in range(NP):
                                nc.tensor.matmul(
                                    yp[:],
                                    lhsT=o_keep[:, kk,
                                                m * 128:(m + 1) * 128],
                                    rhs=wp_sb[:, kk, sl],
                                    start=(kk == 0), stop=False)
                            # bias folded into the accumulation group as a
                            # K=1 rank-1 matmul (ones column x bias row) so
                            # the eviction is a cheap copy, not a TT add
                            nc.tensor.matmul(
                                yp[:], lhsT=ones_sb[:],
                                rhs=bias_sb[0:1, sl],
                                start=False, stop=True)
                            nc.vector.tensor_copy(y_sb[:, sl], yp[:])
                        for dh in range(2):
                            nc.sync.dma_start(
                                y_part[m // 2, dh,
                                       (m % 2) * 128:(m % 2 + 1) * 128, :],
                                y_sb[:, dh * 512:(dh + 1) * 512])
                    entries.append(tile_work)
                    if mi % 2 == 1:
                        entries.append(lambda c=m // 2: rs_out(c))
                return entries

            # entries pulled ahead of each attend block's first S-matmul:
            # queued PE fill work hides the kT/qT eviction latency the
            # S-matmul waits on (PE is strict FIFO), keeping HAM warm in
            # the short early blocks
            LOOKAHEAD = (5, 3, 0, 0)

            for bi in range(len(BLOCKS)):
                q0, qw = BLOCKS[bi]
                kth = (q0 + qw - 1) // QB
                for p in range(NP):
                    drain_until(p, kth)
                    feeder(LOOKAHEAD[min(bi, 3)])
                    o_ps = attend_block(p, bi)
                    finish_pair_blk(p, bi, o_ps)
                # splice the projection right after the current drain
                # position so it runs ASAP (hidden under later attention)
                chain[pos[0]:pos[0]] = [(e, None)
                                        for e in proj_blk_entries(bi)]
            # drain remaining background work (late projection chunks)
            while emit_next():
                pass

    nc.compile()
    return nc


# ---------------------------------------------------------------------------
# host-side sharding + entry point
# ---------------------------------------------------------------------------

_NC_CACHE = {}


def _get_nc():
    if "nc" not in _NC_CACHE:
        _NC_CACHE["nc"] = build_nc()
    return _NC_CACHE["nc"]


def _make_in_maps(x, Wq, Wk, Wv, Wp, bp):
    x = np.asarray(x, dtype=np.float32)
    Wq = np.asarray(Wq, dtype=np.float32)
    Wk = np.asarray(Wk, dtype=np.float32)
    Wv = np.asarray(Wv, dtype=np.float32)
    Wp = np.asarray(Wp, dtype=np.float32)
    bp = np.asarray(bp, dtype=np.float32)

    bf = mybir.dt.np(BF16)
    mask = np.zeros((128, 128), dtype=np.float32)
    k_idx = np.arange(128)[:, None]
    q_idx = np.arange(128)[None, :]
    mask[q_idx >= k_idx] = 1.0
    mask = mask.astype(bf)

    xTs = [np.ascontiguousarray(x[b].T).astype(bf) for b in range(B)]
    WpT = np.ascontiguousarray(Wp.T)
    in_maps = []
    for c in range(N_CORES):
        b, g = c // 2, c % 2
        rows = slice(512 * g, 512 * (g + 1))
        m = {
            "xT": xTs[b],
            "wqT": np.ascontiguousarray(Wq[rows, :].T).astype(bf),
            "wkT": np.ascontiguousarray(Wk[rows, :].T).astype(bf),
            "wvT": np.ascontiguousarray(Wv[rows, :].T).astype(bf),
            "wpT": np.ascontiguousarray(WpT[rows, :]).astype(bf),
            "bias": (bp if g == 0 else np.zeros_like(bp))
                    .reshape(1, D).astype(bf),
            "mask": mask,
        }
        in_maps.append(m)
    return in_maps


def kernel(x, Wq, Wk, Wv, Wp, bp, _trace=False, _mode=None):
    nc = _get_nc()
    in_maps = _make_in_maps(x, Wq, Wk, Wv, Wp, bp)
    res = _run_spmd(nc, in_maps, trace=_trace)
    out = np.empty((B, T, D), dtype=np.float32)
    for b in range(B):
        out[b, :, 0:512] = res.results[2 * b]["yout"].astype(np.float32)
        out[b, :, 512:D] = res.results[2 * b + 1]["yout"].astype(np.float32)
    if _trace:
        kernel.last_results = res
    return out


# revision 41
# speedup vs baseline: 1.0528x; 1.0528x over previous
"""Trainium2 Bass kernel for causal multi-head attention + output projection.

Problem (hardcoded): x[4, 2048, 1024] fp32, 16 heads, head_dim 64, causal,
torch-Linear convention (y = x @ W.T), output projection with bias.

Sharding over 8 NeuronCores: batch (4) x head-group (2 groups of 8 heads).
Core c = (b, g): computes q/k/v for heads [8g, 8g+8) of batch b, causal
attention in the S^T layout (keys on partitions, queries on free dim), a
partial output projection over its own 512 O-dims for all 2048 queries, and
a pairwise ReduceScatter(add) scattered along the OUTPUT-D dimension: core
even ends with the final y[:, 0:512], core odd with y[:, 512:1024], for all
2048 rows.  The host concatenates along D.

Structure (vs the pair-major baseline at 376us; this version ~368us):
  - attention runs QUERY-BLOCK-major (qb outer, pair inner), so after each
    qb all 4 pairs' o_keep rows for that qb exist and the projection + two
    256-row ReduceScatter chunks launch immediately -> 6 of the 8
    collectives hide under later attention (the old version serialized
    ~100us of collectives at the end).
  - input DMAs are batched into 0.5-1MB transfers split across the sync
    (x, token-block chunks) and gpsimd (weights) queues; the old per-128KB
    chunks ran at ~180GB/s on one queue and the ones-column scatter DMA
    (4096 2-byte descriptors, ~34us!) is now a gpsimd memset.
  - PSUM->SBUF evictions all ride the Vector engine (Scalar = exp only;
    measured better than parity-alternating them onto Scalar); projection
    bias is folded into the matmul accumulation group as a K=1
    ones-x-bias-row rank-1 update so its eviction is a plain copy.
  - QKV emission chain is token-block-major (th outer, pair inner) to feed
    the qb-major attention order; projection entries are spliced into the
    chain at the current drain position so they emit during later work.

Measured (HW traces): PE busy ~275us (the critical engine; ~14 GFLOP bf16
vs 78.6 TF/s peak => ~178us ideal + AV's 65/128-partition denominator tax
+ LDWEIGHTS + ~40us HAM cold-clock), ACT/exp ~155-210us, DVE ~80-145us,
8x ReduceScatter ~15us each on a 26GB/s bus.  Rejected experiments: fp8
anywhere (sim rel-err 2.4-5e-2 > 2e-2 gate), 256-wide attention blocks
(HAM oscillation + peer-skewed collectives, +90us), PE-side causal mask
via maskneg@I accumulation (+25us, breaks S-pair co-execution), feeder
front-loading (starves the PE queue head on WAR deps).

Attention per (pair, qb): the two heads share one 2-bank PSUM tile for S^T
(head at col 0 / 512 -> different banks), the two row-tiled (64x128) S
matmuls co-execute on PE tiles (0,0)/(64,0), and the softmax exp for both
heads is ONE ACT instruction on a strided [128, 2, n] view.  Softmax
denominators ride as a ones-column in V (row 64 of the O psum);
normalization = reciprocal + partition-broadcast via a small DRAM bounce on
the gpsimd queue, off the critical path.

All matmul operands are bf16 (~0.5% rel err, same PE throughput as fp32r,
half the DMA/SBUF/collective traffic).  PSUM accumulation is fp32.
"""
import sys
import types
from contextlib import ExitStack

import numpy as np

import concourse.bass as bass
import concourse.mybir as mybir
import concourse.tile as tile
from concourse import bacc, bass_utils

F32 = mybir.dt.float32
BF16 = mybir.dt.bfloat16
AF = mybir.ActivationFunctionType
OP = mybir.AluOpType

import os as _os
_AV_SPLIT = bool(int(_os.environ.get("ATTN_AV_SPLIT", "1")))
_JBATCH = int(_os.environ.get("ATTN_JBATCH", "2"))

B, T, D = 4, 2048, 1024
HG = 8           # heads per core
NP = 4           # head pairs per core
QB = 512         # query block
NQB = T // QB    # 4 query blocks
N_CORES = 8
SCALE = 1.0 / 8.0
MODE = "rs"  # harness compat


# ---------------------------------------------------------------------------
# environment glue
# ---------------------------------------------------------------------------

def _install_ntff_hook():
    if 'antenv.axon_hooks' in sys.modules:
        return
    try:
        from trn_agent_boot.trn_boot import _ntff_profile_via_ctypes
        hook = _ntff_profile_via_ctypes('/opt/axon/libaxon_pjrt.so')
    except Exception:
        hook = None
    mod = types.ModuleType('antenv.axon_hooks')
    mod.get_axon_ntff_profile_hook = lambda: hook
    mod.set_axon_ntff_profile_hook = lambda h: None
    sys.modules['antenv.axon_hooks'] = mod


def _run_spmd(nc, in_maps, trace=False):
    from concourse.bass_interp import get_hw_module
    bass_utils.upload_artifacts = lambda tmpdir: tmpdir
    if trace:
        _install_ntff_hook()
    old_m = nc.m
    nc.m = get_hw_module(nc.m)
    try:
        return bass_utils.run_bass_kernel_spmd(
            nc, in_maps, core_ids=list(range(N_CORES)),
            trace=trace, trace_cores=[0] if trace else None,
        )
    finally:
        nc.m = old_m


# ---------------------------------------------------------------------------
# kernel program
# ---------------------------------------------------------------------------

def build_nc():
    nc = bacc.Bacc("TRN2", target_bir_lowering=False, debug=False,
                   enable_asserts=False, num_devices=N_CORES)
    xT = nc.dram_tensor("xT", [D, T], BF16, kind="ExternalInput").ap()
    wqT = nc.dram_tensor("wqT", [D, 512], BF16, kind="ExternalInput").ap()
    wkT = nc.dram_tensor("wkT", [D, 512], BF16, kind="ExternalInput").ap()
    wvT = nc.dram_tensor("wvT", [D, 512], BF16, kind="ExternalInput").ap()
    wpT = nc.dram_tensor("wpT", [512, D], BF16, kind="ExternalInput").ap()
    bias = nc.dram_tensor("bias", [1, D], BF16, kind="ExternalInput").ap()
    mask = nc.dram_tensor("mask", [128, 128], BF16, kind="ExternalInput").ap()
    snum = nc.dram_tensor("snum", [20, 2 * QB], F32).ap()
    srecd = nc.dram_tensor("srecd", [20, 2 * QB], F32).ap()
    # y_part[c] = partial y rows [256c, +256) split into the two D-halves
    # (scatter dim first) so each ReduceScatter chunk is contiguous
    y_part = nc.dram_tensor("y_part", [T // 256, 2, 256, 512], BF16).ap()
    yred = nc.dram_tensor("yred", [T, 512], BF16).ap()
    yout = nc.dram_tensor("yout", [T, 512], BF16, kind="ExternalOutput").ap()

    with tile.TileContext(nc) as tc, ExitStack() as ctx:
        per = ctx.enter_context(tc.tile_pool(name="per", bufs=1))

        mask_sb = per.tile([128, 128], BF16, tag="mask")
        qT_sb = per.tile([128, NP, T], BF16, tag="qT")
        kT_sb = per.tile([128, NP, T], BF16, tag="kT")
        o_keep = per.tile([128, NP, T], BF16, tag="okeep")
        bias_sb = per.tile([1, D], BF16, tag="brow")
        ones_sb = per.tile([1, 128], BF16, tag="ones")
        wp_sb = per.tile([128, NP, D], BF16, tag="wp")
        wk_sb = per.tile([128, 8, 512], BF16, tag="wk")
        wq_sb = per.tile([128, 8, 512], BF16, tag="wq")
        wv_sb = per.tile([128, 8, 512], BF16, tag="wv")
        x_th = [per.tile([128, 8, QB], BF16, tag=f"x{th}", name=f"x_th{th}")
                for th in range(NQB)]

        xT_r = xT.rearrange("(ko ki) t -> ki ko t", ki=128)

        # batched input loads: first-consumed first; x in token-block
        # chunks on sync (the first attention block only needs tokens
        # 0:512), weights on gpsimd, so the queues stream in parallel
        wkT_r = wkT.rearrange("(ko ki) n -> ki ko n", ki=128)
        nc.gpsimd.dma_start(wk_sb[:, 0:4], wkT_r[:, 0:4])
        nc.sync.dma_start(x_th[0][:, 0:4], xT_r[:, 0:4, 0:QB])
        nc.gpsimd.dma_start(wk_sb[:, 4:8], wkT_r[:, 4:8])
        nc.sync.dma_start(x_th[0][:, 4:8], xT_r[:, 4:8, 0:QB])
        nc.gpsimd.dma_start(wq_sb[:],
                            wqT.rearrange("(ko ki) n -> ki ko n", ki=128))
        nc.sync.dma_start(mask_sb[:], mask[:])
        nc.sync.dma_start(bias_sb[:], bias[:])
        nc.gpsimd.dma_start(wv_sb[:],
                            wvT.rearrange("(ko ki) n -> ki ko n", ki=128))
        for th in range(1, NQB):
            nc.sync.dma_start(x_th[th][:], xT_r[:, :, th * QB:(th + 1) * QB])
        nc.gpsimd.dma_start(wp_sb[:],
                            wpT.rearrange("(ko ki) n -> ki ko n", ki=128))
        nc.gpsimd.memset(ones_sb[:], 1.0)

        def xh(kk, th):
            return x_th[th][:, kk, :]

        with ExitStack() as attn_ctx:
            vpool = attn_ctx.enter_context(tc.tile_pool(name="vpool", bufs=4))
            qkps = attn_ctx.enter_context(
                tc.tile_pool(name="qkps", bufs=2, space="PSUM"))
            sps = attn_ctx.enter_context(
                tc.tile_pool(name="sps", bufs=2, space="PSUM"))
            ops = attn_ctx.enter_context(
                tc.tile_pool(name="ops", bufs=2, space="PSUM"))
            epool = attn_ctx.enter_context(tc.tile_pool(name="epool", bufs=3))
            npool = attn_ctx.enter_context(tc.tile_pool(name="npool", bufs=4))
            ypool = attn_ctx.enter_context(tc.tile_pool(name="ypool", bufs=3))

            v_tiles = {}

            # ----------------------------------------------------------
            # background QKV emission chain, token-block-major: after
            # marker (p, th), pair p's q/k/v for tokens up to 512*(th+1)
            # are fully emitted.
            # ----------------------------------------------------------
            def qkv_chain():
                chain = []
                for th in range(NQB):
                    for p in range(NP):
                        if th == 0:
                            # ones column (softmax denominator rows of the
                            # AV psum) via engine memset -- a scatter DMA
                            # here costs 4096 2-byte descriptors (~34us!)
                            def ones_set(p=p):
                                v_sb = vpool.tile([128, 16, 2, 65], BF16,
                                                  tag="v", name=f"v{p}")
                                v_tiles[p] = v_sb
                                nc.gpsimd.memset(v_sb[:, :, :, 64], 1.0)
                            chain.append((ones_set, None))
                        for wsb, dst in ((wk_sb, kT_sb), (wq_sb, qT_sb)):
                            box = {}

                            def fill(half, box=box, wsb=wsb, th=th, p=p):
                                if half == 0:
                                    box["pt"] = qkps.tile([128, QB], F32,
                                                          tag="pt", name="pt")
                                pt = box["pt"]
                                for kk in range(4 * half, 4 * half + 4):
                                    nc.tensor.matmul(
                                        pt[:],
                                        lhsT=wsb[:, kk, p * 128:(p + 1) * 128],
                                        rhs=xh(kk, th),
                                        start=(kk == 0), stop=(kk == 7))

                            def evict(box=box, dst=dst, th=th, p=p):
                                nc.vector.tensor_copy(
                                    dst[:, p, th * QB:(th + 1) * QB],
                                    box["pt"][:])
                            chain.append((lambda f=fill: f(0), None))
                            chain.append((lambda f=fill: f(1), None))
                            chain.append((evict, None))
                        # V for key blocks 4*th .. 4*th+3 (token-major)
                        box = {}

                        def vfill(sub, box=box, th=th, p=p):
                            if sub == 0:
                                box["pt"] = qkps.tile([128, QB], F32,
                                                      tag="pt", name="pt")
                            pt = box["pt"]
                            for kk in range(8):
                                nc.tensor.matmul(
                                    pt[:, sub * 128:(sub + 1) * 128],
                                    lhsT=xh(kk, th)[:,
                                                    sub * 128:(sub + 1) * 128],
                                    rhs=wv_sb[:, kk, p * 128:(p + 1) * 128],
                                    start=(kk == 0), stop=(kk == 7))

                        def vevict(box=box, th=th, p=p):
                            nc.vector.tensor_copy(
                                v_tiles[p][:, 4 * th:4 * th + 4, :, 0:64],
                                box["pt"][:].rearrange(
                                    "q (m h d) -> q m h d", m=4, h=2))
                        for sub in range(4):
                            chain.append((lambda f=vfill, s=sub: f(s), None))
                        chain.append((vevict, (p, th)))
                return chain

            chain = qkv_chain()
            pos = [0]
            emitted = {}

            def emit_next():
                if pos[0] >= len(chain):
                    return False
                fn, marker = chain[pos[0]]
                pos[0] += 1
                fn()
                if marker is not None:
                    emitted[marker[0]] = marker[1]
                return True

            def drain_until(p, th):
                while emitted.get(p, -1) < th:
                    if not emit_next():
                        raise RuntimeError("qkv chain exhausted early")

            def feeder(k):
                for _ in range(k):
                    if not emit_next():
                        return

            # ----------------------------------------------------------
            # attention + normalization, over generalized query blocks
            # (q0, qw): qb0-2 run 512-wide; the LAST 512 queries run as
            # two 256-wide blocks so the final projection + ReduceScatter
            # chunk shrinks (the exposed tail halves).
            # ----------------------------------------------------------
            BLOCKS = [(0, QB), (QB, QB), (2 * QB, QB), (3 * QB, QB)]

            def attend_block(p, bi):
                q0, qw = BLOCKS[bi]
                jmax = (q0 + qw) // 128
                o_ps = [ops.tile([65, QB], F32, tag="o", name=f"o{hl}")
                        for hl in range(2)]
                for j0 in range(0, jmax, _JBATCH):
                    batch = range(j0, min(j0 + _JBATCH, jmax))
                    s_tiles = {}
                    e_tiles = {}
                    for j in batch:
                        qs = max(0, 128 * j - q0)
                        s_t = sps.tile([128, 2 * QB], F32, tag="s",
                                       name=f"s{j}")
                        s_tiles[j] = s_t
                        for hl in range(2):
                            pb = 64 * hl
                            # head hl at column hl*QB: each head's S stays
                            # in its OWN PSUM bank (start=True clears the
                            # whole bank's has_written bits)
                            nc.tensor.matmul(
                                s_t[:, hl * QB + qs:hl * QB + qw],
                                lhsT=kT_sb[pb:pb + 64, p,
                                           j * 128:(j + 1) * 128],
                                rhs=qT_sb[pb:pb + 64, p,
                                          q0 + qs:q0 + qw],
                                start=True, stop=True)
                    for j in batch:
                        qs = max(0, 128 * j - q0)
                        e_t = epool.tile([128, 2, QB], BF16, tag="e",
                                         name=f"e{j}")
                        e_tiles[j] = e_t
                        s_v = s_tiles[j].rearrange("q (h n) -> q h n", h=2)
                        nc.scalar.activation(e_t[:, :, qs:qw],
                                             s_v[:, :, qs:qw],
                                             AF.Exp, scale=SCALE)
                        if 128 * j >= q0:
                            nc.vector.tensor_tensor(
                                e_t[:, :, qs:qs + 128],
                                e_t[:, :, qs:qs + 128],
                                mask_sb[:, None, :]
                                .broadcast_to([128, 2, 128]),
                                OP.mult)
                    feeder(1)
                    for j in batch:
                        qs = max(0, 128 * j - q0)
                        e_t = e_tiles[j]
                        last = (j == jmax - 1)
                        for hl in range(2):
                            if 128 * j >= q0 and _AV_SPLIT and j > 0:
                                if qs + 128 < qw:
                                    nc.tensor.matmul(
                                        o_ps[hl][:, qs + 128:qw],
                                        lhsT=v_tiles[p][:, j, hl, :],
                                        rhs=e_t[:, hl, qs + 128:qw],
                                        start=(j == 0), stop=False,
                                        skip_group_check=True)
                                nc.tensor.matmul(
                                    o_ps[hl][:, qs:qs + 128],
                                    lhsT=v_tiles[p][:, j, hl, :],
                                    rhs=e_t[:, hl, qs:qs + 128],
                                    start=(j == 0), stop=last,
                                    skip_group_check=True)
                            else:
                                nc.tensor.matmul(
                                    o_ps[hl][:, qs:qw],
                                    lhsT=v_tiles[p][:, j, hl, :],
                                    rhs=e_t[:, hl, qs:qw],
                                    start=(j == 0), stop=last,
                                    skip_group_check=True)
                    feeder(1)
                return o_ps

            def finish_pair_blk(p, bi, o_ps):
                """Evict + normalize both heads of the pair for this query
                block.  Denominators of both heads ride ONE DRAM bounce:
                write [2, qw], reload spread as [64, 2qw/64], reciprocal
                (few cols -> fast), write back, one broadcast load for both
                partition halves.  DMAs go on the gpsimd queue to keep the
                other queues clear."""
                q0, qw = BLOCKS[bi]
                row = p * len(BLOCKS) + bi
                stmp = npool.tile([1, 2 * QB], F32, tag="st", name="stmp")
                for hl in range(2):
                    nc.vector.tensor_copy(stmp[0:1, hl * qw:(hl + 1) * qw],
                                          o_ps[hl][64:65, 0:qw])
                nc.gpsimd.dma_start(snum[row:row + 1, 0:2 * qw],
                                    stmp[0:1, 0:2 * qw])
                st64 = npool.tile([64, 2 * QB // 64], F32, tag="sp",
                                  name="st64")
                nb = 2 * qw // 64
                nc.gpsimd.dma_start(
                    st64[:, 0:nb],
                    snum[row, 0:2 * qw].rearrange("(a b) -> a b", a=64))
                nc.vector.reciprocal(st64[:, 0:nb], st64[:, 0:nb])
                nc.gpsimd.dma_start(
                    srecd[row, 0:2 * qw].rearrange("(a b) -> a b", a=64),
                    st64[:, 0:nb])
                bcr = npool.tile([128, QB], F32, tag="bcr", name="bcr")
                for hl in range(2):
                    pb = 64 * hl
                    nc.gpsimd.dma_start(
                        bcr[pb:pb + 64, 0:qw],
                        srecd[row][None, hl * qw:(hl + 1) * qw]
                        .broadcast_to([64, qw]))
                    dst = o_keep[pb:pb + 64, p, q0:q0 + qw]
                    nc.vector.tensor_copy(dst, o_ps[hl][0:64, 0:qw])
                    nc.vector.tensor_tensor(dst, dst, bcr[pb:pb + 64, 0:qw],
                                            OP.mult)

            # ----------------------------------------------------------
            # projection for a query block: y rows [q0, q0+qw), all 1024
            # output cols, then pairwise ReduceScatter along D in 256-row
            # chunks.  Needs o_keep of ALL pairs for the block -> entries
            # spliced into the chain right after the block's last pair.
            # ----------------------------------------------------------
            def rs_out(c):
                nc.gpsimd.collective_compute(
                    "ReduceScatter", OP.add,
                    replica_groups=[[0, 1], [2, 3], [4, 5], [6, 7]],
                    ins=[y_part[c]],
                    outs=[yred[c * 256:(c + 1) * 256, :]],
                )
                nc.sync.dma_start(yout[c * 256:(c + 1) * 256, :],
                                  yred[c * 256:(c + 1) * 256, :])

            def proj_blk_entries(bi):
                q0, qw = BLOCKS[bi]
                entries = []
                for mi in range(qw // 128):
                    m = q0 // 128 + mi

                    def tile_work(m=m):
                        y_sb = ypool.tile([128, D], BF16, tag="y",
                                          name="y_sb")
                        for nch in range(2):
                            sl = slice(nch * 512, (nch + 1) * 512)
                            yp = qkps.tile([128, QB], F32, tag="pt",
                                           name="yp")
                            for kk in range(NP):
                                nc.tensor.matmul(
                                    yp[:],
                                    lhsT=o_keep[:, kk,
                                                m * 128:(m + 1) * 128],
                                    rhs=wp_sb[:, kk, sl],
                                    start=(kk == 0), stop=False)
                            # bias folded into the accumulation group as a
                            # K=1 rank-1 matmul (ones column x bias row) so
                            # the eviction is a cheap copy, not a TT add
                            nc.tensor.matmul(
                                yp[:], lhsT=ones_sb[:],
                                rhs=bias_sb[0:1, sl],
                                start=False, stop=True)
                            nc.vector.tensor_copy(y_sb[:, sl], yp[:])
                        for dh in range(2):
                            nc.sync.dma_start(
                                y_part[m // 2, dh,
                                       (m % 2) * 128:(m % 2 + 1) * 128, :],
                                y_sb[:, dh * 512:(dh + 1) * 512])
                    entries.append(tile_work)
                    if mi % 2 == 1:
                        entries.append(lambda c=m // 2: rs_out(c))
                return entries

            # entries pulled ahead of each attend block's first S-matmul:
            # queued PE fill work hides the kT/qT eviction latency the
            # S-matmul waits on (PE is strict FIFO), keeping HAM warm in
            # the short early blocks
            LOOKAHEAD = (5, 3, 0, 0)

            for bi in range(len(BLOCKS)):
                q0, qw = BLOCKS[bi]
                kth = (q0 + qw - 1) // QB
                for p in range(NP):
                    drain_until(p, kth)
                    feeder(LOOKAHEAD[min(bi, 3)])
                    o_ps = attend_block(p, bi)
                    finish_pair_blk(p, bi, o_ps)
                # splice the projection right after the current drain
                # position so it runs ASAP (hidden under later attention)
                chain[pos[0]:pos[0]] = [(e, None)
                                        for e in proj_blk_entries(bi)]
            # drain remaining background work (late projection chunks)
            while emit_next():
                pass

    nc.compile()
    return nc


# ---------------------------------------------------------------------------
# host-side sharding + entry point
# ---------------------------------------------------------------------------

_NC_CACHE = {}


def _get_nc():
    if "nc" not in _NC_CACHE:
        _NC_CACHE["nc"] = build_nc()
    return _NC_CACHE["nc"]


def _make_in_maps(x, Wq, Wk, Wv, Wp, bp):
    x = np.asarray(x, dtype=np.float32)
    Wq = np.asarray(Wq, dtype=np.float32)
    Wk = np.asarray(Wk, dtype=np.float32)
    Wv = np.asarray(Wv, dtype=np.float32)
    Wp = np.asarray(Wp, dtype=np.float32)
    bp = np.asarray(bp, dtype=np.float32)

    bf = mybir.dt.np(BF16)
    mask = np.zeros((128, 128), dtype=np.float32)
    k_idx = np.arange(128)[:, None]
    q_idx = np.arange(128)[None, :]
    mask[q_idx >= k_idx] = 1.0
    mask = mask.astype(bf)

    xTs = [np.ascontiguousarray(x[b].T).astype(bf) for b in range(B)]
    WpT = np.ascontiguousarray(Wp.T)
    in_maps = []
    for c in range(N_CORES):
        b, g = c // 2, c % 2
        rows = slice(512 * g, 512 * (g + 1))
        m = {
            "xT": xTs[b],
            "wqT": np.ascontiguousarray(Wq[rows, :].T).astype(bf),
            "wkT": np.ascontiguousarray(Wk[rows, :].T).astype(bf),
            "wvT": np.ascontiguousarray(Wv[rows, :].T).astype(bf),
            "wpT": np.ascontiguousarray(WpT[rows, :]).astype(bf),
            "bias": (bp if g == 0 else np.zeros_like(bp))
                    .reshape(1, D).astype(bf),
            "mask": mask,
        }
        in_maps.append(m)
    return in_maps


def kernel(x, Wq, Wk, Wv, Wp, bp, _trace=False, _mode=None):
    nc = _get_nc()
    in_maps = _make_in_maps(x, Wq, Wk, Wv, Wp, bp)
    res = _run_spmd(nc, in_maps, trace=_trace)
    out = np.empty((B, T, D), dtype=np.float32)
    for b in range(B):
        out[b, :, 0:512] = res.results[2 * b]["yout"].astype(np.float32)
        out[b, :, 512:D] = res.results[2 * b + 1]["yout"].astype(np.float32)
    if _trace:
        kernel.last_results = res
    return out


# revision 44
# speedup vs baseline: 1.0584x; 1.0053x over previous
"""Trainium2 Bass kernel for causal multi-head attention + output projection.

Problem (hardcoded): x[4, 2048, 1024] fp32, 16 heads, head_dim 64, causal,
torch-Linear convention (y = x @ W.T), output projection with bias.

Sharding over 8 NeuronCores: batch (4) x head-group (2 groups of 8 heads).
Core c = (b, g): computes q/k/v for heads [8g, 8g+8) of batch b, causal
attention in the S^T layout (keys on partitions, queries on free dim), a
partial output projection over its own 512 O-dims for all 2048 queries, and
a pairwise ReduceScatter(add) scattered along the OUTPUT-D dimension: core
even ends with the final y[:, 0:512], core odd with y[:, 512:1024], for all
2048 rows.  The host concatenates along D.

Structure (vs the pair-major baseline at 376us; this version ~355-364us):
  - attention runs QUERY-BLOCK-major (qb outer, pair inner), so after each
    qb all 4 pairs' o_keep rows for that qb exist and the projection + two
    256-row ReduceScatter chunks launch immediately -> 6 of the 8
    collectives hide under later attention (the old version serialized
    ~100us of collectives at the end).
  - input DMAs are batched into 0.5-1MB transfers split across the sync
    (x, token-block chunks) and gpsimd (weights) queues; the old per-128KB
    chunks ran at ~180GB/s on one queue and the ones-column scatter DMA
    (4096 2-byte descriptors, ~34us!) is now a gpsimd memset.
  - PSUM->SBUF evictions all ride the Vector engine (Scalar = exp only;
    measured better than parity-alternating them onto Scalar); projection
    bias is folded into the matmul accumulation group as a K=1
    ones-x-bias-row rank-1 update so its eviction is a plain copy.
  - QKV emission chain is token-block-major (th outer, pair inner) to feed
    the qb-major attention order; projection entries are spliced into the
    chain at the current drain position so they emit during later work.

Measured (HW traces): PE busy ~275us (the critical engine; ~14 GFLOP bf16
vs 78.6 TF/s peak => ~178us ideal + AV's 65/128-partition denominator tax
+ LDWEIGHTS + ~40us HAM cold-clock), ACT/exp ~155-210us, DVE ~80-145us,
8x ReduceScatter ~15us each on a 26GB/s bus.  Rejected experiments: fp8
anywhere (sim rel-err 2.4-5e-2 > 2e-2 gate), 256-wide attention blocks
(HAM oscillation + peer-skewed collectives, +90us), PE-side causal mask
via maskneg@I accumulation (+25us, breaks S-pair co-execution), feeder
front-loading (starves the PE queue head on WAR deps).

Attention per (pair, qb): the two heads share one 2-bank PSUM tile for S^T
(head at col 0 / 512 -> different banks), the two row-tiled (64x128) S
matmuls co-execute on PE tiles (0,0)/(64,0), and the softmax exp for both
heads is ONE ACT instruction on a strided [128, 2, n] view.  Softmax
denominators ride as a ones-column in V (row 64 of the O psum);
normalization = reciprocal + partition-broadcast via a small DRAM bounce,
alternating the gpsimd/sync queues by pair parity so the LAST pair's bounce
(which gates the block's projection) never queues behind the other pairs'.

All matmul operands are bf16 (~0.5% rel err, same PE throughput as fp32r,
half the DMA/SBUF/collective traffic).  PSUM accumulation is fp32.
"""
import sys
import types
from contextlib import ExitStack

import numpy as np

import concourse.bass as bass
import concourse.mybir as mybir
import concourse.tile as tile
from concourse import bacc, bass_utils

F32 = mybir.dt.float32
BF16 = mybir.dt.bfloat16
AF = mybir.ActivationFunctionType
OP = mybir.AluOpType

import os as _os
_AV_SPLIT = bool(int(_os.environ.get("ATTN_AV_SPLIT", "1")))
_JBATCH = int(_os.environ.get("ATTN_JBATCH", "2"))

B, T, D = 4, 2048, 1024
HG = 8           # heads per core
NP = 4           # head pairs per core
QB = 512         # query block
NQB = T // QB    # 4 query blocks
N_CORES = 8
SCALE = 1.0 / 8.0
MODE = "rs"  # harness compat


# ---------------------------------------------------------------------------
# environment glue
# ---------------------------------------------------------------------------

def _install_ntff_hook():
    if 'antenv.axon_hooks' in sys.modules:
        return
    try:
        from trn_agent_boot.trn_boot import _ntff_profile_via_ctypes
        hook = _ntff_profile_via_ctypes('/opt/axon/libaxon_pjrt.so')
    except Exception:
        hook = None
    mod = types.ModuleType('antenv.axon_hooks')
    mod.get_axon_ntff_profile_hook = lambda: hook
    mod.set_axon_ntff_profile_hook = lambda h: None
    sys.modules['antenv.axon_hooks'] = mod


def _run_spmd(nc, in_maps, trace=False):
    from concourse.bass_interp import get_hw_module
    bass_utils.upload_artifacts = lambda tmpdir: tmpdir
    if trace:
        _install_ntff_hook()
    old_m = nc.m
    nc.m = get_hw_module(nc.m)
    try:
        return bass_utils.run_bass_kernel_spmd(
            nc, in_maps, core_ids=list(range(N_CORES)),
            trace=trace, trace_cores=[0] if trace else None,
        )
    finally:
        nc.m = old_m


# ---------------------------------------------------------------------------
# kernel program
# ---------------------------------------------------------------------------

def build_nc():
    nc = bacc.Bacc("TRN2", target_bir_lowering=False, debug=False,
                   enable_asserts=False, num_devices=N_CORES)
    xT = nc.dram_tensor("xT", [D, T], BF16, kind="ExternalInput").ap()
    wqT = nc.dram_tensor("wqT", [D, 512], BF16, kind="ExternalInput").ap()
    wkT = nc.dram_tensor("wkT", [D, 512], BF16, kind="ExternalInput").ap()
    wvT = nc.dram_tensor("wvT", [D, 512], BF16, kind="ExternalInput").ap()
    wpT = nc.dram_tensor("wpT", [512, D], BF16, kind="ExternalInput").ap()
    bias = nc.dram_tensor("bias", [1, D], BF16, kind="ExternalInput").ap()
    mask = nc.dram_tensor("mask", [128, 128], BF16, kind="ExternalInput").ap()
    snum = nc.dram_tensor("snum", [20, 2 * QB], F32).ap()
    srecd = nc.dram_tensor("srecd", [20, 2 * QB], F32).ap()
    # y_part[c] = partial y rows [256c, +256) split into the two D-halves
    # (scatter dim first) so each ReduceScatter chunk is contiguous
    y_part = nc.dram_tensor("y_part", [T // 256, 2, 256, 512], BF16).ap()
    yred = nc.dram_tensor("yred", [T, 512], BF16).ap()
    yout = nc.dram_tensor("yout", [T, 512], BF16, kind="ExternalOutput").ap()

    with tile.TileContext(nc) as tc, ExitStack() as ctx:
        per = ctx.enter_context(tc.tile_pool(name="per", bufs=1))

        mask_sb = per.tile([128, 128], BF16, tag="mask")
        qT_sb = per.tile([128, NP, T], BF16, tag="qT")
        kT_sb = per.tile([128, NP, T], BF16, tag="kT")
        o_keep = per.tile([128, NP, T], BF16, tag="okeep")
        bias_sb = per.tile([1, D], BF16, tag="brow")
        ones_sb = per.tile([1, 128], BF16, tag="ones")
        wp_sb = per.tile([128, NP, D], BF16, tag="wp")
        wk_sb = per.tile([128, 8, 512], BF16, tag="wk")
        wq_sb = per.tile([128, 8, 512], BF16, tag="wq")
        wv_sb = per.tile([128, 8, 512], BF16, tag="wv")
        x_th = [per.tile([128, 8, QB], BF16, tag=f"x{th}", name=f"x_th{th}")
                for th in range(NQB)]

        xT_r = xT.rearrange("(ko ki) t -> ki ko t", ki=128)

        # batched input loads: first-consumed first; x in token-block
        # chunks on sync (the first attention block only needs tokens
        # 0:512), weights on gpsimd, so the queues stream in parallel
        wkT_r = wkT.rearrange("(ko ki) n -> ki ko n", ki=128)
        nc.gpsimd.dma_start(wk_sb[:, 0:4], wkT_r[:, 0:4])
        nc.sync.dma_start(x_th[0][:, 0:4], xT_r[:, 0:4, 0:QB])
        nc.gpsimd.dma_start(wk_sb[:, 4:8], wkT_r[:, 4:8])
        nc.sync.dma_start(x_th[0][:, 4:8], xT_r[:, 4:8, 0:QB])
        nc.gpsimd.dma_start(wq_sb[:],
                            wqT.rearrange("(ko ki) n -> ki ko n", ki=128))
        nc.sync.dma_start(mask_sb[:], mask[:])
        nc.sync.dma_start(bias_sb[:], bias[:])
        nc.gpsimd.dma_start(wv_sb[:],
                            wvT.rearrange("(ko ki) n -> ki ko n", ki=128))
        for th in range(1, NQB):
            nc.sync.dma_start(x_th[th][:], xT_r[:, :, th * QB:(th + 1) * QB])
        nc.gpsimd.dma_start(wp_sb[:],
                            wpT.rearrange("(ko ki) n -> ki ko n", ki=128))
        nc.gpsimd.memset(ones_sb[:], 1.0)

        def xh(kk, th):
            return x_th[th][:, kk, :]

        with ExitStack() as attn_ctx:
            vpool = attn_ctx.enter_context(tc.tile_pool(name="vpool", bufs=4))
            qkps = attn_ctx.enter_context(
                tc.tile_pool(name="qkps", bufs=2, space="PSUM"))
            sps = attn_ctx.enter_context(
                tc.tile_pool(name="sps", bufs=2, space="PSUM"))
            ops = attn_ctx.enter_context(
                tc.tile_pool(name="ops", bufs=2, space="PSUM"))
            epool = attn_ctx.enter_context(tc.tile_pool(name="epool", bufs=4))
            npool = attn_ctx.enter_context(tc.tile_pool(name="npool", bufs=4))
            ypool = attn_ctx.enter_context(tc.tile_pool(name="ypool", bufs=3))

            v_tiles = {}

            # ----------------------------------------------------------
            # background QKV emission chain, token-block-major: after
            # marker (p, th), pair p's q/k/v for tokens up to 512*(th+1)
            # are fully emitted.
            # ----------------------------------------------------------
            def qkv_chain():
                chain = []
                for th in range(NQB):
                    for p in range(NP):
                        if th == 0:
                            # ones column (softmax denominator rows of the
                            # AV psum) via engine memset -- a scatter DMA
                            # here costs 4096 2-byte descriptors (~34us!)
                            def ones_set(p=p):
                                v_sb = vpool.tile([128, 16, 2, 65], BF16,
                                                  tag="v", name=f"v{p}")
                                v_tiles[p] = v_sb
                                nc.gpsimd.memset(v_sb[:, :, :, 64], 1.0)
                            chain.append((ones_set, None))
                        for wsb, dst in ((wk_sb, kT_sb), (wq_sb, qT_sb)):
                            box = {}

                            def fill(half, box=box, wsb=wsb, th=th, p=p):
                                if half == 0:
                                    box["pt"] = qkps.tile([128, QB], F32,
                                                          tag="pt", name="pt")
                                pt = box["pt"]
                                for kk in range(4 * half, 4 * half + 4):
                                    nc.tensor.matmul(
                                        pt[:],
                                        lhsT=wsb[:, kk, p * 128:(p + 1) * 128],
                                        rhs=xh(kk, th),
                                        start=(kk == 0), stop=(kk == 7))

                            def evict(box=box, dst=dst, th=th, p=p):
                                nc.vector.tensor_copy(
                                    dst[:, p, th * QB:(th + 1) * QB],
                                    box["pt"][:])
                            chain.append((lambda f=fill: f(0), None))
                            chain.append((lambda f=fill: f(1), None))
                            chain.append((evict, None))
                        # V for key blocks 4*th .. 4*th+3 (token-major)
                        box = {}

                        def vfill(sub, box=box, th=th, p=p):
                            if sub == 0:
                                box["pt"] = qkps.tile([128, QB], F32,
                                                      tag="pt", name="pt")
                            pt = box["pt"]
                            for kk in range(8):
                                nc.tensor.matmul(
                                    pt[:, sub * 128:(sub + 1) * 128],
                                    lhsT=xh(kk, th)[:,
                                                    sub * 128:(sub + 1) * 128],
                                    rhs=wv_sb[:, kk, p * 128:(p + 1) * 128],
                                    start=(kk == 0), stop=(kk == 7))

                        def vevict(box=box, th=th, p=p):
                            nc.vector.tensor_copy(
                                v_tiles[p][:, 4 * th:4 * th + 4, :, 0:64],
                                box["pt"][:].rearrange(
                                    "q (m h d) -> q m h d", m=4, h=2))
                        for sub in range(4):
                            chain.append((lambda f=vfill, s=sub: f(s), None))
                        chain.append((vevict, (p, th)))
                return chain

            chain = qkv_chain()
            pos = [0]
            emitted = {}

            def emit_next():
                if pos[0] >= len(chain):
                    return False
                fn, marker = chain[pos[0]]
                pos[0] += 1
                fn()
                if marker is not None:
                    emitted[marker[0]] = marker[1]
                return True

            def drain_until(p, th):
                while emitted.get(p, -1) < th:
                    if not emit_next():
                        raise RuntimeError("qkv chain exhausted early")

            def feeder(k):
                for _ in range(k):
                    if not emit_next():
                        return

            # ----------------------------------------------------------
            # attention + normalization, over generalized query blocks
            # (q0, qw): qb0-2 run 512-wide; the LAST 512 queries run as
            # two 256-wide blocks so the final projection + ReduceScatter
            # chunk shrinks (the exposed tail halves).
            # ----------------------------------------------------------
            BLOCKS = [(0, QB), (QB, QB), (2 * QB, QB), (3 * QB, QB)]

            def attend_block(p, bi):
                q0, qw = BLOCKS[bi]
                jmax = (q0 + qw) // 128
                o_ps = [ops.tile([65, QB], F32, tag="o", name=f"o{hl}")
                        for hl in range(2)]
                for j0 in range(0, jmax, _JBATCH):
                    batch = range(j0, min(j0 + _JBATCH, jmax))
                    s_tiles = {}
                    e_tiles = {}
                    for j in batch:
                        qs = max(0, 128 * j - q0)
                        s_t = sps.tile([128, 2 * QB], F32, tag="s",
                                       name=f"s{j}")
                        s_tiles[j] = s_t
                        for hl in range(2):
                            pb = 64 * hl
                            # head hl at column hl*QB: each head's S stays
                            # in its OWN PSUM bank (start=True clears the
                            # whole bank's has_written bits)
                            nc.tensor.matmul(
                                s_t[:, hl * QB + qs:hl * QB + qw],
                                lhsT=kT_sb[pb:pb + 64, p,
                                           j * 128:(j + 1) * 128],
                                rhs=qT_sb[pb:pb + 64, p,
                                          q0 + qs:q0 + qw],
                                start=True, stop=True)
                    for j in batch:
                        qs = max(0, 128 * j - q0)
                        e_t = epool.tile([128, 2, QB], BF16, tag="e",
                                         name=f"e{j}")
                        e_tiles[j] = e_t
                        s_v = s_tiles[j].rearrange("q (h n) -> q h n", h=2)
                        nc.scalar.activation(e_t[:, :, qs:qw],
                                             s_v[:, :, qs:qw],
                                             AF.Exp, scale=SCALE)
                        if 128 * j >= q0:
                            nc.vector.tensor_tensor(
                                e_t[:, :, qs:qs + 128],
                                e_t[:, :, qs:qs + 128],
                                mask_sb[:, None, :]
                                .broadcast_to([128, 2, 128]),
                                OP.mult)
                    feeder(1)
                    for j in batch:
                        qs = max(0, 128 * j - q0)
                        e_t = e_tiles[j]
                        last = (j == jmax - 1)
                        for hl in range(2):
                            if 128 * j >= q0 and _AV_SPLIT and j > 0:
                                if qs + 128 < qw:
                                    nc.tensor.matmul(
                                        o_ps[hl][:, qs + 128:qw],
                                        lhsT=v_tiles[p][:, j, hl, :],
                                        rhs=e_t[:, hl, qs + 128:qw],
                                        start=(j == 0), stop=False,
                                        skip_group_check=True)
                                nc.tensor.matmul(
                                    o_ps[hl][:, qs:qs + 128],
                                    lhsT=v_tiles[p][:, j, hl, :],
                                    rhs=e_t[:, hl, qs:qs + 128],
                                    start=(j == 0), stop=last,
                                    skip_group_check=True)
                            else:
                                nc.tensor.matmul(
                                    o_ps[hl][:, qs:qw],
                                    lhsT=v_tiles[p][:, j, hl, :],
                                    rhs=e_t[:, hl, qs:qw],
                                    start=(j == 0), stop=last,
                                    skip_group_check=True)
                    feeder(1)
                return o_ps

            def finish_pair_blk(p, bi, o_ps):
                """Evict + normalize both heads of the pair for this query
                block.  Denominators of both heads ride ONE DRAM bounce:
                write [2, qw], reload spread as [64, 2qw/64], reciprocal
                (few cols -> fast), write back, one broadcast load for both
                partition halves.  DMAs go on the gpsimd queue to keep the
                other queues clear."""
                q0, qw = BLOCKS[bi]
                row = p * len(BLOCKS) + bi
                # alternate the bounce DMA queue by pair so the last pair's
                # normalization (which gates the projection) doesn't sit
                # behind the other pairs' bounces in one FIFO
                dq = nc.sync if p % 2 else nc.gpsimd
                stmp = npool.tile([1, 2 * QB], F32, tag="st", name="stmp")
                for hl in range(2):
                    nc.vector.tensor_copy(stmp[0:1, hl * qw:(hl + 1) * qw],
                                          o_ps[hl][64:65, 0:qw])
                dq.dma_start(snum[row:row + 1, 0:2 * qw],
                                    stmp[0:1, 0:2 * qw])
                st64 = npool.tile([64, 2 * QB // 64], F32, tag="sp",
                                  name="st64")
                nb = 2 * qw // 64
                dq.dma_start(
                    st64[:, 0:nb],
                    snum[row, 0:2 * qw].rearrange("(a b) -> a b", a=64))
                nc.vector.reciprocal(st64[:, 0:nb], st64[:, 0:nb])
                dq.dma_start(
                    srecd[row, 0:2 * qw].rearrange("(a b) -> a b", a=64),
                    st64[:, 0:nb])
                bcr = npool.tile([128, QB], F32, tag="bcr", name="bcr")
                for hl in range(2):
                    pb = 64 * hl
                    dq.dma_start(
                        bcr[pb:pb + 64, 0:qw],
                        srecd[row][None, hl * qw:(hl + 1) * qw]
                        .broadcast_to([64, qw]))
                    dst = o_keep[pb:pb + 64, p, q0:q0 + qw]
                    nc.vector.tensor_copy(dst, o_ps[hl][0:64, 0:qw])
                    nc.vector.tensor_tensor(dst, dst, bcr[pb:pb + 64, 0:qw],
                                            OP.mult)

            # ----------------------------------------------------------
            # projection for a query block: y rows [q0, q0+qw), all 1024
            # output cols, then pairwise ReduceScatter along D in 256-row
            # chunks.  Needs o_keep of ALL pairs for the block -> entries
            # spliced into the chain right after the block's last pair.
            # ----------------------------------------------------------
            def rs_out(c):
                nc.gpsimd.collective_compute(
                    "ReduceScatter", OP.add,
                    replica_groups=[[0, 1], [2, 3], [4, 5], [6, 7]],
                    ins=[y_part[c]],
                    outs=[yred[c * 256:(c + 1) * 256, :]],
                )
                nc.sync.dma_start(yout[c * 256:(c + 1) * 256, :],
                                  yred[c * 256:(c + 1) * 256, :])

            def proj_blk_entries(bi):
                q0, qw = BLOCKS[bi]
                entries = []
                for mi in range(qw // 128):
                    m = q0 // 128 + mi

                    def tile_work(m=m):
                        y_sb = ypool.tile([128, D], BF16, tag="y",
                                          name="y_sb")
                        for nch in range(2):
                            sl = slice(nch * 512, (nch + 1) * 512)
                            yp = qkps.tile([128, QB], F32, tag="pt",
                                           name="yp")
                            for kk in range(NP):
                                nc.tensor.matmul(
                                    yp[:],
                                    lhsT=o_keep[:, kk,
                                                m * 128:(m + 1) * 128],
                                    rhs=wp_sb[:, kk, sl],
                                    start=(kk == 0), stop=False)
                            # bias folded into the accumulation group as a
                            # K=1 rank-1 matmul (ones column x bias row) so
                            # the eviction is a cheap copy, not a TT add
                            nc.tensor.matmul(
                                yp[:], lhsT=ones_sb[:],
                                rhs=bias_sb[0:1, sl],
                                start=False, stop=True)
                            nc.vector.tensor_copy(y_sb[:, sl], yp[:])
                        for dh in range(2):
                            nc.sync.dma_start(
                                y_part[m // 2, dh,
                                       (m % 2) * 128:(m % 2 + 1) * 128, :],
                                y_sb[:, dh * 512:(dh + 1) * 512])
                    entries.append(tile_work)
                    if mi % 2 == 1:
                        entries.append(lambda c=m // 2: rs_out(c))
                return entries

            # entries pulled ahead of each attend block's first S-matmul:
            # queued PE fill work hides the kT/qT eviction latency the
            # S-matmul waits on (PE is strict FIFO), keeping HAM warm in
            # the short early blocks
            LOOKAHEAD = (5, 3, 0, 0)

            for bi in range(len(BLOCKS)):
                q0, qw = BLOCKS[bi]
                kth = (q0 + qw - 1) // QB
                for p in range(NP):
                    drain_until(p, kth)
                    feeder(LOOKAHEAD[min(bi, 3)])
                    o_ps = attend_block(p, bi)
                    finish_pair_blk(p, bi, o_ps)
                # splice the projection right after the current drain
                # position so it runs ASAP (hidden under later attention)
                chain[pos[0]:pos[0]] = [(e, None)
                                        for e in proj_blk_entries(bi)]
            # drain remaining background work (late projection chunks)
            while emit_next():
                pass

    nc.compile()
    return nc


# ---------------------------------------------------------------------------
# host-side sharding + entry point
# ---------------------------------------------------------------------------

_NC_CACHE = {}


def _get_nc():
    if "nc" not in _NC_CACHE:
        _NC_CACHE["nc"] = build_nc()
    return _NC_CACHE["nc"]


def _make_in_maps(x, Wq, Wk, Wv, Wp, bp):
    x = np.asarray(x, dtype=np.float32)
    Wq = np.asarray(Wq, dtype=np.float32)
    Wk = np.asarray(Wk, dtype=np.float32)
    Wv = np.asarray(Wv, dtype=np.float32)
    Wp = np.asarray(Wp, dtype=np.float32)
    bp = np.asarray(bp, dtype=np.float32)

    bf = mybir.dt.np(BF16)
    mask = np.zeros((128, 128), dtype=np.float32)
    k_idx = np.arange(128)[:, None]
    q_idx = np.arange(128)[None, :]
    mask[q_idx >= k_idx] = 1.0
    mask = mask.astype(bf)

    xTs = [np.ascontiguousarray(x[b].T).astype(bf) for b in range(B)]
    WpT = np.ascontiguousarray(Wp.T)
    in_maps = []
    for c in range(N_CORES):
        b, g = c // 2, c % 2
        rows = slice(512 * g, 512 * (g + 1))
        m = {
            "xT": xTs[b],
            "wqT": np.ascontiguousarray(Wq[rows, :].T).astype(bf),
            "wkT": np.ascontiguousarray(Wk[rows, :].T).astype(bf),
            "wvT": np.ascontiguousarray(Wv[rows, :].T).astype(bf),
            "wpT": np.ascontiguousarray(WpT[rows, :]).astype(bf),
            "bias": (bp if g == 0 else np.zeros_like(bp))
                    .reshape(1, D).astype(bf),
            "mask": mask,
        }
        in_maps.append(m)
    return in_maps


def kernel(x, Wq, Wk, Wv, Wp, bp, _trace=False, _mode=None):
    nc = _get_nc()
    in_maps = _make_in_maps(x, Wq, Wk, Wv, Wp, bp)
    res = _run_spmd(nc, in_maps, trace=_trace)
    out = np.empty((B, T, D), dtype=np.float32)
    for b in range(B):
        out[b, :, 0:512] = res.results[2 * b]["yout"].astype(np.float32)
        out[b, :, 512:D] = res.results[2 * b + 1]["yout"].astype(np.float32)
    if _trace:
        kernel.last_results = res
    return out
